# revision 15
# baseline (speedup 1.0000x reference)
"""Trainium2 Bass kernel for nn_AttentionBlock (sparse_attention).

Reference computation per batch b (channels-first x[b]: [C=512, T=4096]):
    xt = x[b].T                                  # [T, C]
    q = xt @ Wq.T + bq ; k = xt @ Wk.T + bk      # [T, 512]
    v = xt @ Wv.T + bv                           # [T, 512]
    S = q @ k.T / sqrt(512), causal (j <= i)     # [T, T]
    P = softmax(S, axis=QUERY i)  (per-column normalization)
    act = P @ v                                  # [T, 512]
    out[b] = concat(x[b], act.T, axis=0)         # [1024, T]

Sharding: pure data-parallel over batch B=8 across the 8 NeuronCores
(one batch per core, no collectives).

Per-core design (everything fp8e4m3 + DoubleRow on TensorE):
  1. Q^T,K^T projections from host-cast x8/w8 (fp8, c-chunk-paired for
     DoubleRow).  1/sqrt(512) folded into Wq,bq,Wk,bk host-side as
     512**-0.25 on each side.  PSUM f32 -> bias-add -> qt8/kt8 (fp8,
     kk-chunk-paired layout for the score matmuls).
  2. Phase 1 per key-strip jc (128 keys on partitions): V chunk
     projection (fp8 DR matmuls + DVE bias -> v16), score strips
     ST[j,i] via fp8 DR matmuls from the diagonal to T, additive
     causal mask on the diagonal 128x128, exp on ScalarE with a
     per-strip shift (bias AP) writing P~ directly into an
     SBUF-resident fp8 strip; ScalarE accum_out produces the Z row
     sums for free.  Z floored (fp8 overflow seatbelt), reciprocal,
     folded into v8 (fp8, pair-of-strips layout).
  3. After every 4 strips, act block ib: PSUM-accumulated fp8 DR
     matmuls act^T[v,i] = sum_j V'[j,v] P~[j,i] reading P~ straight
     from SBUF; evacuate to out rows 512..1023.
  4. x passthrough: DRAM->DRAM DMA copies of the host-uploaded f32 x
     (upload is not metered; engines untouched), spread across phase 1
     so they ride idle DMA slots.

P~ fp8 dynamic range: per-strip exp shift c_jc (host cvec, bias AP).
c=-4.6 keeps exp(s+c) in fp8 normal range for long strips; the last
strip (few terms, tiny Z) uses c=-0.55 so v/Z stays well under fp8
max 240.  Z floored at 0.025 as an overflow seatbelt.  Validated in
numpy vs the reference: global rel err ~1.1e-2 (gate 2e-2).
"""

import math

import numpy as np

import concourse.bass as bass
import concourse.mybir as mybir
from concourse import bacc, tile
from concourse.bass_utils import run_bass_kernel_spmd

P = 128
C = 512
T = 4096
KDIM = 512
VDIM = 512
NCC = C // P      # 4 contraction chunks over channels
NKK = KDIM // P   # 4 chunks of head dim
NTC = T // P      # 32 key strips of 128
NIB = T // 512    # 8 i-blocks of 512
F8 = mybir.dt.float8e4
F16 = mybir.dt.float16
F32 = mybir.dt.float32
SHIFT_MAIN = -4.6
SHIFT_LAST = -0.55
Z_FLOOR = 0.025   # keeps |v/Z| <= ~220 < fp8e4 max 240
MASK_NEG = -10000.0

_CACHE = {}


def _ts(i, size):
    return slice(i * size, (i + 1) * size)


def build_nc():
    nc = bacc.Bacc(
        "TRN2",
        target_bir_lowering=False,
        debug=False,
        num_devices=8,
    )

    x32_d = nc.declare_dram_parameter("x32", [C, T], F32, isOutput=False)
    x8_d = nc.declare_dram_parameter("x8", [C, T], F8, isOutput=False)
    wq8_d = nc.declare_dram_parameter("wq8", [P, NCC * KDIM], F8, isOutput=False)
    wk8_d = nc.declare_dram_parameter("wk8", [P, NCC * KDIM], F8, isOutput=False)
    wv8_d = nc.declare_dram_parameter("wv8", [P, NCC * VDIM], F8, isOutput=False)
    bq_d = nc.declare_dram_parameter("bq", [P, NKK], F32, isOutput=False)
    bk_d = nc.declare_dram_parameter("bk", [P, NKK], F32, isOutput=False)
    bv_d = nc.declare_dram_parameter("bv", [P, VDIM], F32, isOutput=False)
    mask_d = nc.declare_dram_parameter("mask", [P, P], F32, isOutput=False)
    cvec_d = nc.declare_dram_parameter("cvec", [P, NTC], F32, isOutput=False)
    out_d = nc.declare_dram_parameter("out", [C + VDIM, T], F32, isOutput=True)

    def pair3(ap2d):
        # [128, 2*n] -> [128, 2, n] u-major view for DoubleRow operands
        return ap2d.rearrange("p (u n) -> p u n", u=2)

    with tile.TileContext(nc) as tc:
        from contextlib import ExitStack

        with ExitStack() as ctx:
            singles = ctx.enter_context(tc.tile_pool(name="singles", bufs=1))

            def single(shape, dtype, tag):
                return singles.tile(shape, dtype, name=tag, tag=tag)

            # x8 split into 8 tiles [h c-pair][g col-group of 1024] so the
            # first QK matmuls unblock after one small DMA, not 0.5MB x4
            NG = 4
            x8_s = [
                [single([P, 2 * 1024], F8, f"x8s{h}g{g}") for g in range(NG)]
                for h in range(2)
            ]
            wq8_s = single([P, NCC * KDIM], F8, "wq8s")
            wk8_s = single([P, NCC * KDIM], F8, "wk8s")
            wv8_s = single([P, NCC * VDIM], F8, "wv8s")
            bq_s = single([P, NKK], F32, "bqs")
            bk_s = single([P, NKK], F32, "bks")
            bv_s = single([P, VDIM], F32, "bvs")
            mask_s = single([P, P], F32, "masks")
            cvec_s = single([P, NTC], F32, "cvecs")
            qt8_s = [single([P, 2 * T], F8, f"qt8s{h}") for h in range(2)]
            kt8_s = [single([P, 2 * T], F8, f"kt8s{h}") for h in range(2)]
            # P~ strips, SBUF-resident: pair m holds strips (2m, 2m+1),
            # covering absolute i in [a0, T), a0 = 512*(m//2)
            lens = [T - 512 * (m // 2) for m in range(NTC // 2)]
            pt8_s = [
                single([P, 2 * lens[m]], F8, f"pt8s{m}") for m in range(NTC // 2)
            ]
            v8_s = [single([P, 2 * VDIM], F8, f"v8s{m}") for m in range(NTC // 2)]
            zr_s = single([P, NTC], F32, "zrs")

            # ---- input DMAs (first QK deps first: wq8 + x8 g-pieces) ----
            nc.sync.dma_start(out=wq8_s, in_=wq8_d[:, :])
            for g in range(NG):
                for c in range(NCC):
                    nc.sync.dma_start(
                        out=x8_s[c // 2][g][:, _ts(c % 2, 1024)],
                        in_=x8_d[_ts(c, P), _ts(g, 1024)],
                    )
                if g == 0:
                    nc.sync.dma_start(out=bq_s, in_=bq_d[:, :])
                    nc.sync.dma_start(out=wk8_s, in_=wk8_d[:, :])
            nc.sync.dma_start(out=bk_s, in_=bk_d[:, :])
            nc.sync.dma_start(out=wv8_s, in_=wv8_d[:, :])
            nc.sync.dma_start(out=bv_s, in_=bv_d[:, :])
            nc.sync.dma_start(out=mask_s, in_=mask_d[:, :])
            nc.sync.dma_start(out=cvec_s, in_=cvec_d[:, :])

            s_ps = ctx.enter_context(
                tc.tile_pool(name="s_ps", bufs=4, space="PSUM")
            )
            act_ps = ctx.enter_context(
                tc.tile_pool(name="act_ps", bufs=1, space="PSUM")
            )
            v16_pool = ctx.enter_context(tc.tile_pool(name="v16", bufs=4))
            zp_pool = ctx.enter_context(tc.tile_pool(name="zp", bufs=4))
            ob_pool = ctx.enter_context(tc.tile_pool(name="ob", bufs=4))

            # ---- Q^T / K^T projections ----
            # out[kk-chunk, i] = sum_c W'[c, kk].T @ x[c, i], fp8 DR pairs
            for which in range(2):  # 0 = Q, 1 = K
                w_s = (wq8_s, wk8_s)[which]
                b_s = (bq_s, bk_s)[which]
                dst = (qt8_s, kt8_s)[which]
                for kk in range(NKK):
                    for ib in range(NIB):
                        ps = s_ps.tile([P, 512], F32, tag="sps", name="ps_qk")
                        for h in range(2):
                            lhs3 = pair3(w_s[:, _ts(h, 2 * KDIM)])[
                                :, :, _ts(kk, P)
                            ]
                            rhs3 = pair3(x8_s[h][ib // 2])[
                                :, :, _ts(ib % 2, 512)
                            ]
                            nc.tensor.matmul(
                                ps,
                                lhsT=lhs3,
                                rhs=rhs3,
                                start=(h == 0),
                                stop=(h == 1),
                                perf_mode=mybir.MatmulPerfMode.DoubleRow,
                            )
                        dst_ap = dst[kk // 2][
                            :, (kk % 2) * T + ib * 512 : (kk % 2) * T + ib * 512 + 512
                        ]
                        if which == 0:
                            nc.scalar.activation(
                                dst_ap,
                                ps,
                                mybir.ActivationFunctionType.Identity,
                                bias=b_s[:, kk : kk + 1],
                                scale=1.0,
                            )
                        else:
                            nc.vector.tensor_scalar_add(
                                dst_ap, ps, b_s[:, kk : kk + 1]
                            )

            # ---- Phase 1 (scores+softmax) and phase 2 (act) interleaved ----
            def emit_act_block(ib):
                nm = 2 * (ib + 1)
                pss = [
                    act_ps.tile([P, 512], F32, tag=f"aps{v}", name=f"aps{v}")
                    for v in range(4)
                ]
                for m in range(nm):
                    off = ib * 512 - 512 * (m // 2)
                    rhs3 = pt8_s[m].rearrange("p (u n) -> p u n", u=2)[
                        :, :, off : off + 512
                    ]
                    for vc in range(4):
                        lhs3 = pair3(v8_s[m])[:, :, _ts(vc, P)]
                        nc.tensor.matmul(
                            pss[vc],
                            lhsT=lhs3,
                            rhs=rhs3,
                            start=(m == 0),
                            stop=(m == nm - 1),
                            perf_mode=mybir.MatmulPerfMode.DoubleRow,
                        )
                for vc in range(4):
                    ob = ob_pool.tile([P, 512], F32, tag="ob", name="ob")
                    if vc == 0:
                        nc.scalar.copy(ob, pss[vc])
                    else:
                        nc.vector.tensor_copy(ob, pss[vc])
                    nc.sync.dma_start(
                        out=out_d[C + vc * P : C + (vc + 1) * P, _ts(ib, 512)],
                        in_=ob,
                    )

            # x passthrough: DRAM -> DRAM copies, spread across phase 1
            def emit_xpass_piece(pi):
                c, half = pi // 2, pi % 2
                nc.sync.dma_start(
                    out=out_d[_ts(c, P), _ts(half, 2048)],
                    in_=x32_d[_ts(c, P), _ts(half, 2048)],
                )

            for jc in range(NTC):
                i0 = P * jc
                a0 = 512 * (jc // 4)
                m, u = jc // 2, jc % 2
                r = jc % 4
                ln = lens[m]

                # V chunk jc: [t-chunk, v] = sum_c x[c, t].T @ Wv[c, v]
                ps_v = s_ps.tile([P, 512], F32, tag="sps", name="ps_v")
                for h in range(2):
                    lhs3 = pair3(x8_s[h][jc // 8])[:, :, _ts(jc % 8, P)]
                    rhs3 = pair3(wv8_s[:, _ts(h, 2 * VDIM)])
                    nc.tensor.matmul(
                        ps_v,
                        lhsT=lhs3,
                        rhs=rhs3,
                        start=(h == 0),
                        stop=(h == 1),
                        perf_mode=mybir.MatmulPerfMode.DoubleRow,
                    )
                v16 = v16_pool.tile([P, 512], F16, tag="v16", name="v16")
                nc.vector.tensor_add(v16, ps_v, bv_s)

                if r > 0:
                    # zero the never-written corner [a0, i0)
                    nc.vector.memset(pt8_s[m][:, u * ln : u * ln + (i0 - a0)], 0.0)

                starts = [i0] + list(range(a0 + 512, T, 512))
                nch = len(starts)
                zp = zp_pool.tile([P, NIB], F32, tag="zp", name="zp")
                for ci, a in enumerate(starts):
                    b = a0 + 512 * (ci + 1)
                    w = b - a
                    ps = s_ps.tile([P, 512], F32, tag="sps", name="ps_s")
                    for h in range(2):
                        lhs3 = pair3(kt8_s[h])[:, :, i0 : i0 + P]
                        rhs3 = pair3(qt8_s[h])[:, :, a:b]
                        nc.tensor.matmul(
                            ps[:, 0:w],
                            lhsT=lhs3,
                            rhs=rhs3,
                            start=(h == 0),
                            stop=(h == 1),
                            perf_mode=mybir.MatmulPerfMode.DoubleRow,
                        )
                    if ci == 0:
                        nc.vector.tensor_add(
                            ps[:, 0:P], ps[:, 0:P], mask_s
                        )
                    base = u * ln + (a - a0)
                    nc.scalar.activation(
                        pt8_s[m][:, base : base + w],
                        ps[:, 0:w],
                        mybir.ActivationFunctionType.Exp,
                        bias=cvec_s[:, jc : jc + 1],
                        scale=1.0,
                        accum_out=zp[:, ci : ci + 1],
                    )
                z = zp_pool.tile([P, 1], F32, tag="zf", name="z")
                nc.vector.reduce_sum(
                    z, zp[:, 0:nch], axis=mybir.AxisListType.X
                )
                nc.vector.tensor_scalar_max(z, z, Z_FLOOR)
                nc.vector.reciprocal(zr_s[:, jc : jc + 1], z)
                # fold 1/Z_j into V rows (partition j), fp16 -> fp8
                nc.vector.tensor_scalar_mul(
                    v8_s[m][:, _ts(u, VDIM)], v16, zr_s[:, jc : jc + 1]
                )
                if 2 <= jc < 10:
                    emit_xpass_piece(jc - 2)
                if r == 3:
                    emit_act_block(jc // 4)

    nc.compile()
    return nc


def _host_inputs(x, Wq, bq, Wk, bk, Wv, bv):
    import ml_dtypes

    f8 = ml_dtypes.float8_e4m3  # TRN FP8_EXP4 bit layout for |v| <= 240
    c4 = float(C) ** 0.25

    def wpack(wt):  # [C, K] -> [128, NCC*K] chunk-major fp8
        return np.ascontiguousarray(
            wt.reshape(NCC, P, -1).transpose(1, 0, 2).reshape(P, -1)
        ).astype(f8)

    wq8 = wpack(Wq.T / c4)
    wk8 = wpack(Wk.T / c4)
    wv8 = wpack(Wv.T)
    bq_h = np.ascontiguousarray((bq / c4).reshape(NKK, P).T).astype(np.float32)
    bk_h = np.ascontiguousarray((bk / c4).reshape(NKK, P).T).astype(np.float32)
    bv_h = np.ascontiguousarray(np.tile(bv.astype(np.float32), (P, 1)))
    r = np.arange(P)
    mask = np.where(r[None, :] >= r[:, None], 0.0, MASK_NEG).astype(np.float32)
    cvec = np.full((P, NTC), SHIFT_MAIN, np.float32)
    cvec[:, NTC - 1] = SHIFT_LAST
    in_maps = []
    for b in range(x.shape[0]):
        xb = np.ascontiguousarray(x[b]).astype(np.float32)
        in_maps.append(
            {
                "x32": xb,
                "x8": xb.astype(f8),
                "wq8": wq8,
                "wk8": wk8,
                "wv8": wv8,
                "bq": bq_h,
                "bk": bk_h,
                "bv": bv_h,
                "mask": mask,
                "cvec": cvec,
            }
        )
    return in_maps


def kernel(x, Wq, bq, Wk, bk, Wv, bv, _trace=False):
    import time as _time

    x = np.asarray(x, dtype=np.float32)
    if "nc" not in _CACHE:
        t0 = _time.time()
        _CACHE["nc"] = build_nc()
        print(f"[kernel] build_nc done in {_time.time() - t0:.1f}s", flush=True)
    nc = _CACHE["nc"]
    in_maps = _host_inputs(
        x,
        np.asarray(Wq, np.float32),
        np.asarray(bq, np.float32),
        np.asarray(Wk, np.float32),
        np.asarray(bk, np.float32),
        np.asarray(Wv, np.float32),
        np.asarray(bv, np.float32),
    )
    t0 = _time.time()
    res = run_bass_kernel_spmd(
        nc, in_maps, core_ids=list(range(8)), trace=_trace
    )
    print(f"[kernel] run done in {_time.time() - t0:.1f}s", flush=True)
    _CACHE["last_result"] = res
    out = np.stack([r["out"] for r in res.results]).astype(np.float32)
    return out
